# revision 1
# baseline (speedup 1.0000x reference)
"""Self-contained TRN2 Bass kernel for the Chemprop D-MPNN layer.

kernel(**inputs) takes the FULL problem inputs (edge_feats [500000,128] f32,
node_feats [50000,1] f32, W [128,128], b [128], edge_index [2,500000] i64,
rev_index [500000] i64) and returns the full [500000,128] f32 output, running
SPMD on 8 NeuronCores.

Strategy: nodes split into 128-node windows, 49 windows per core (dest- and
src-sharded phases share the same windows so the per-core node table stays in
SBUF). Phase A builds transformed node sums Aw = (segment_sum(relu(ef)) @ W.T)
per window via one-hot matmuls with PSUM accumulation. Phase C computes
out[j,e] = Aw[src[e]] - (W.T.T @ relu(ef[rev[e]])) + b via one-hot gather
matmul + accumulated halo matmul, writing the output transposed; the host
inverse-permutes. No collectives, no indirect DMA; fp16 streams, f32r/fp16
matmuls (~4e-4 rel err).
"""

import math
import numpy as np

import concourse.bass as bass
import concourse.bacc as bacc
import concourse.mybir as mybir
import concourse.tile as tile

F32 = mybir.dt.float32
F32R = mybir.dt.float32r
BF16 = mybir.dt.bfloat16
FP16 = mybir.dt.float16
P = 128


def cdiv(a, b):
    return -(-a // b)


class Prep:
    pass


def prep_inputs(edge_feats, W, b, edge_index, rev_index, V, n_cores=8,
                tile_e=512):
    E, D = edge_feats.shape
    assert D == P
    src = np.asarray(edge_index[0], dtype=np.int64)
    dest = np.asarray(edge_index[1], dtype=np.int64)
    rev = np.asarray(rev_index, dtype=np.int64)

    WPC = cdiv(V, n_cores * P)
    NW = n_cores * WPC

    ef = np.asarray(edge_feats, dtype=np.float32)

    def bin_edges(keys):
        win = keys // P
        order = np.argsort(win, kind="stable")
        starts = np.searchsorted(win[order], np.arange(NW + 1))
        return order, starts

    ordA, stA = bin_edges(dest)
    ordC, stC = bin_edges(src)

    cntA = np.zeros((n_cores, WPC), dtype=np.int64)
    cntC = np.zeros((n_cores, WPC), dtype=np.int64)
    for k in range(n_cores):
        for j in range(WPC):
            w = k * WPC + j
            cntA[k, j] = stA[w + 1] - stA[w]
            cntC[k, j] = stC[w + 1] - stC[w]
    T_A = np.maximum(-(-cntA.max(axis=0) // P), 1)
    E_C = (-(-cntC.max(axis=0) // P)) * P

    tiles_C = []
    for j in range(WPC):
        w = int(E_C[j])
        sizes = []
        while w > 0:
            s = min(tile_e, w)
            sizes.append(s)
            w -= s
        tiles_C.append(sizes)

    NA = int(T_A.sum()) * P
    NC = int(E_C.sum())

    per_core = []
    for k in range(n_cores):
        idsA = np.full(NA, -1, dtype=np.int64)
        dlocA = np.full(NA, -1.0, dtype=np.float32)
        posA = 0
        for j in range(WPC):
            w = k * WPC + j
            ids = ordA[stA[w]:stA[w + 1]]
            n = len(ids)
            idsA[posA:posA + n] = ids
            dlocA[posA:posA + n] = (dest[ids] - w * P).astype(np.float32)
            posA += T_A[j] * P
        idsC = np.full(NC, -1, dtype=np.int64)
        slocC = np.full(NC, -1.0, dtype=np.float32)
        posC = 0
        for j in range(WPC):
            w = k * WPC + j
            ids = ordC[stC[w]:stC[w + 1]]
            n = len(ids)
            idsC[posC:posC + n] = ids
            slocC[posC:posC + n] = (src[ids] - w * P).astype(np.float32)
            posC += int(E_C[j])

        rowsA = np.where(idsA[:, None] >= 0, ef[np.maximum(idsA, 0)], 0.0)
        efA_T = np.ascontiguousarray(
            rowsA.reshape(NA // P, P, D).transpose(1, 0, 2)
            .reshape(P, NA).astype(np.float16))
        dlocA_m = np.ascontiguousarray(dlocA.reshape(NA // P, P).T)

        hrows = np.where(idsC[:, None] >= 0, ef[rev[np.maximum(idsC, 0)]], 0.0)
        haloT = np.ascontiguousarray(hrows.T.astype(np.float16))

        per_core.append(dict(
            efA=efA_T, dlocA=dlocA_m, haloT=haloT,
            slocC=np.ascontiguousarray(slocC[None, :].astype(np.float16)),
            idsC=idsC,
        ))

    cfg = Prep()
    cfg.WPC, cfg.NA, cfg.NC = WPC, NA, NC
    cfg.T_A = [int(x) for x in T_A]
    cfg.tiles_C = tiles_C
    cfg.n_cores = n_cores
    cfg.V, cfg.E = V, E

    Wt = np.asarray(W, np.float32).T
    consts = dict(
        Wt=np.ascontiguousarray(Wt),
        negWt=np.ascontiguousarray((-Wt).astype(np.float16)),
        b_col=np.ascontiguousarray(np.asarray(b, np.float32)[:, None]),
        iota_row=np.ascontiguousarray(
            np.tile(np.arange(P, dtype=np.float32)[None, :], (P, 1))),
        iota_col=np.ascontiguousarray(np.arange(P, dtype=np.float32)[:, None]),
    )
    return cfg, per_core, consts


def build_kernel(cfg):
    nc = bacc.Bacc("TRN2", target_bir_lowering=False, debug=False,
                   num_devices=cfg.n_cores)
    WPC, NA, NC = cfg.WPC, cfg.NA, cfg.NC

    efA_d = nc.dram_tensor("efA", [P, NA], FP16, kind="ExternalInput")
    dlocA_d = nc.dram_tensor("dlocA", [P, NA // P], F32, kind="ExternalInput")
    haloT_d = nc.dram_tensor("haloT", [P, NC], FP16, kind="ExternalInput")
    slocC_d = nc.dram_tensor("slocC", [1, NC], FP16, kind="ExternalInput")
    Wt_d = nc.dram_tensor("Wt", [P, P], F32R, kind="ExternalInput")
    negWt_d = nc.dram_tensor("negWt", [P, P], FP16, kind="ExternalInput")
    b_d = nc.dram_tensor("b_col", [P, 1], F32, kind="ExternalInput")
    iota_row_d = nc.dram_tensor("iota_row", [P, P], F32, kind="ExternalInput")
    iota_col_d = nc.dram_tensor("iota_col", [P, 1], F32, kind="ExternalInput")
    out_d = nc.dram_tensor("outT", [P, NC], FP16, kind="ExternalOutput")

    maxEA = max(t * P for t in cfg.T_A)
    maxTA = max(cfg.T_A)
    maxEC = max((sum(t) for t in cfg.tiles_C if t), default=P)
    SB = 4  # S-build batch (chunks per is_equal)

    with tile.TileContext(nc) as tc:
        with (
            tc.tile_pool(name="const", bufs=1) as cpool,
            tc.tile_pool(name="table", bufs=WPC) as tpool,
            tc.tile_pool(name="sa", bufs=6) as sa,
            tc.tile_pool(name="sc", bufs=5) as sc,
            tc.tile_pool(name="wk", bufs=6) as wk,
            tc.tile_pool(name="psA", bufs=1, space="PSUM") as psA,
            tc.tile_pool(name="psT", bufs=1, space="PSUM") as psT,
            tc.tile_pool(name="psO", bufs=4, space="PSUM") as psO,
            tc.tile_pool(name="psB", bufs=2, space="PSUM") as psB,
        ):
            wt_t = cpool.tile([P, P], F32R)
            nc.sync.dma_start(out=wt_t[:], in_=Wt_d[:])
            nwt_t = cpool.tile([P, P], FP16)
            nc.sync.dma_start(out=nwt_t[:], in_=negWt_d[:])
            b_t = cpool.tile([P, 1], F32)
            nc.sync.dma_start(out=b_t[:], in_=b_d[:])
            iota_r = cpool.tile([P, P], F32)
            nc.sync.dma_start(out=iota_r[:], in_=iota_row_d[:])
            iota_c = cpool.tile([P, 1], F32)
            nc.sync.dma_start(out=iota_c[:], in_=iota_col_d[:])
            ones_c = cpool.tile([1, P], FP16)
            nc.vector.memset(ones_c[:], 1.0)

            table = {}
            startA = [0] * WPC
            p = 0
            for j in range(WPC):
                startA[j] = p
                p += cfg.T_A[j]
            startC = [0] * WPC
            p = 0
            for j in range(WPC):
                startC[j] = p
                p += sum(cfg.tiles_C[j])

            def emit_A(j):
                tch = cfg.T_A[j]
                ew = tch * P
                posA = startA[j]
                ef_t = sa.tile([P, maxEA], FP16, tag="ef", name=f"efa{j}")
                nc.sync.dma_start(out=ef_t[:, :ew],
                                  in_=efA_d[:, posA * P: posA * P + ew])
                dl_t = wk.tile([P, maxTA], F32, tag="dloc", name=f"dl{j}")
                nc.sync.dma_start(out=dl_t[:, :tch],
                                  in_=dlocA_d[:, posA: posA + tch])
                nc.scalar.activation(ef_t[:, :ew], ef_t[:, :ew],
                                     mybir.ActivationFunctionType.Relu)
                ps = psA.tile([P, P], F32, tag="psA", name=f"psa{j}")
                for c0 in range(0, tch, SB):
                    g = min(SB, tch - c0)
                    s4_t = wk.tile([P, SB * P], FP16, tag="smat",
                                   name=f"s4_{j}_{c0}")
                    nc.vector.tensor_tensor(
                        out=s4_t[:, :g * P].rearrange("p (a n) -> p a n", a=g),
                        in0=dl_t[:, c0:c0 + g].to_broadcast([P, g, P]),
                        in1=iota_r[:].rearrange("p (a n) -> p a n", a=1)
                            .to_broadcast([P, g, P]),
                        op=mybir.AluOpType.is_equal)
                    for ci in range(g):
                        c = c0 + ci
                        nc.tensor.matmul(out=ps[:],
                                         lhsT=ef_t[:, c * P:(c + 1) * P],
                                         rhs=s4_t[:, ci * P:(ci + 1) * P],
                                         start=(c == 0), stop=(c == tch - 1))
                tdT_t = wk.tile([P, P], F32R, tag="tdT", name=f"tdt{j}")
                nc.scalar.activation(tdT_t[:], ps[:],
                                     mybir.ActivationFunctionType.Copy)
                pst = psT.tile([P, P], F32, tag="psT", name=f"pst{j}")
                nc.tensor.matmul(out=pst[:], lhsT=tdT_t[:], rhs=wt_t[:],
                                 start=True, stop=True)
                aw_t = tpool.tile([P, P], FP16, tag="tab", name=f"aw{j}")
                nc.scalar.activation(aw_t[:], pst[:],
                                     mybir.ActivationFunctionType.Copy)
                table[j] = aw_t

            def emit_C(j):
                sizes = cfg.tiles_C[j]
                ew = sum(sizes)
                if ew == 0:
                    return
                posC = startC[j]
                halo_t = sc.tile([P, maxEC], FP16, tag="halo", name=f"hal{j}")
                nc.sync.dma_start(out=halo_t[:, :ew],
                                  in_=haloT_d[:, posC: posC + ew])
                sl_t = sc.tile([1, maxEC], FP16, tag="sloc", bufs=3,
                               name=f"sl{j}")
                nc.sync.dma_start(out=sl_t[:, :ew],
                                  in_=slocC_d[:, posC: posC + ew])
                nc.scalar.activation(halo_t[:, :ew], halo_t[:, :ew],
                                     mybir.ActivationFunctionType.Relu)
                ot_t = sc.tile([P, maxEC], FP16, tag="outt", name=f"ot{j}")
                off = 0
                ti = 0
                for wdt in sizes:
                    pb = psB.tile([P, 512], F32, tag="pb", name=f"pb{j}_{off}")
                    nc.tensor.matmul(out=pb[:, :wdt], lhsT=ones_c[:],
                                     rhs=sl_t[:1, off:off + wdt],
                                     start=True, stop=True)
                    s3_t = wk.tile([P, 512], FP16, tag="s3",
                                   name=f"s3_{j}_{off}")
                    nc.vector.tensor_scalar(
                        out=s3_t[:, :wdt], in0=pb[:, :wdt],
                        scalar1=iota_c[:, :1], scalar2=None,
                        op0=mybir.AluOpType.is_equal)
                    po = psO.tile([P, 512], F32, tag="po", name=f"po{j}_{off}")
                    nc.tensor.matmul(out=po[:, :wdt], lhsT=table[j][:],
                                     rhs=s3_t[:, :wdt], start=True,
                                     stop=False, skip_group_check=True)
                    nc.tensor.matmul(out=po[:, :wdt], lhsT=nwt_t[:],
                                     rhs=halo_t[:, off:off + wdt], start=False,
                                     stop=True, skip_group_check=True)
                    if ti % 3 == 2:
                        nc.scalar.add(ot_t[:, off:off + wdt], po[:, :wdt],
                                      b_t[:, :1])
                    else:
                        nc.vector.tensor_scalar(
                            out=ot_t[:, off:off + wdt], in0=po[:, :wdt],
                            scalar1=b_t[:, :1], scalar2=None,
                            op0=mybir.AluOpType.add)
                    ti += 1
                    off += wdt
                nc.sync.dma_start(out=out_d[:, posC: posC + ew],
                                  in_=ot_t[:, :ew])

            # interleave A and C emission so every engine's queue mixes both
            LAG = 3
            for j in range(WPC):
                emit_A(j)
                if j >= LAG:
                    emit_C(j - LAG)
            for j in range(max(0, WPC - LAG), WPC):
                emit_C(j)

    nc.compile()
    return nc


def run(edge_feats, node_feats, W, b, edge_index, rev_index, n_cores=8,
        trace=False):
    from concourse import bass_utils
    V = node_feats.shape[0]
    E, D = edge_feats.shape
    cfg, per_core, consts = prep_inputs(edge_feats, W, b, edge_index,
                                        rev_index, V, n_cores=n_cores)
    nc = build_kernel(cfg)
    in_maps = []
    for k in range(n_cores):
        m = dict(per_core[k])
        m.pop("idsC")
        m.update(consts)
        in_maps.append(m)
    res = bass_utils.run_bass_kernel_spmd(
        nc, in_maps, core_ids=list(range(n_cores)), trace=trace)
    out = np.empty((E, D), dtype=np.float32)
    for k in range(n_cores):
        ids = per_core[k]["idsC"]
        valid = ids >= 0
        out[ids[valid]] = res.results[k]["outT"][:, valid].T.astype(np.float32)
    return out, res


_NCORES = 8


def kernel(edge_feats, node_feats, W, b, edge_index, rev_index):
    from concourse import bass_utils
    edge_feats = np.asarray(edge_feats, dtype=np.float32)
    node_feats = np.asarray(node_feats)
    V = node_feats.shape[0]
    E, D = edge_feats.shape
    cfg, per_core, consts = prep_inputs(edge_feats, W, b, edge_index,
                                        rev_index, V, n_cores=_NCORES)
    nc = build_kernel(cfg)
    in_maps = []
    for k in range(_NCORES):
        m = dict(per_core[k])
        m.pop("idsC")
        m.update(consts)
        in_maps.append(m)
    res = bass_utils.run_bass_kernel_spmd(
        nc, in_maps, core_ids=list(range(_NCORES)), trace=False)
    out = np.empty((E, D), dtype=np.float32)
    for k in range(_NCORES):
        ids = per_core[k]["idsC"]
        valid = ids >= 0
        out[ids[valid]] = res.results[k]["outT"][:, valid].T.astype(np.float32)
    return out

